# revision 30
# baseline (speedup 1.0000x reference)
import sys

if "/opt/trn_rl_repo" not in sys.path:
    sys.path.insert(0, "/opt/trn_rl_repo")

import numpy as np

import bass_rust
import concourse.bass as bass
import concourse.tile as tile
from concourse import mybir
from concourse.bass_utils import run_bass_kernel_spmd

# Problem constants (nn_PixelDSNTDoubleEval): B,C,H,W = 32,2,512,512 fp32,
# sharded batch-wise across 8 cores -> 4 samples (8 heatmaps) per core.
B, C, H, W = 32, 2, 512, 512
N_CORES = 8
BPC = B // N_CORES          # batches per core
HM = BPC * C                # heatmaps per core
RPP = H // 128              # rows per partition (4)

TRACE = False
LAST_RESULTS = None

_NC_CACHE = {}


def _build_bass(reps=1, mode="full"):
    nc = bass.Bass()
    inp = nc.declare_dram_parameter("input", [BPC, C, H, W], mybir.dt.float32, isOutput=False)
    tgt = nc.declare_dram_parameter("target", [BPC, C, H, W], mybir.dt.float32, isOutput=False)
    # psums row 0 = column sums of exp; row 1 = (row+1)-weighted column sums
    psums_d = nc.declare_dram_parameter("psums", [2, HM * W], mybir.dt.float32, isOutput=True)
    tmax_d = nc.declare_dram_parameter("tmax", [128, HM], mybir.dt.float32, isOutput=True)
    tidx_d = nc.declare_dram_parameter("tidx", [128, HM], mybir.dt.uint32, isOutput=True)

    with tile.TileContext(nc) as tc:
        with (
            tc.tile_pool(name="big", bufs=(4 if mode == "dmabig" else HM)) as big,
            tc.tile_pool(name="small", bufs=HM) as small,
            tc.tile_pool(name="psum", bufs=2, space="PSUM") as psump,
            tc.tile_pool(name="singles", bufs=1) as singles,
        ):
            # stationary weights: per t-block a (128,2) pair [1, 4p+t+1].
            # Built entirely on gpsimd, then relayed through one ACT copy so
            # matmuls only ever wait on the ACT semaphore (TPB instructions
            # support a single sync-wait slot).
            wt_i = singles.tile([128, 2 * RPP], mybir.dt.float32)
            nc.gpsimd.memset(wt_i, 1.0)
            for t in range(RPP):
                nc.gpsimd.iota(
                    wt_i[:, 2 * t + 1:2 * t + 2],
                    pattern=[[1, 1]],
                    base=t + 1,
                    channel_multiplier=RPP,
                    allow_small_or_imprecise_dtypes=True,
                )
            wt = singles.tile([128, 2 * RPP], mybir.dt.float32)
            nc.scalar.copy(out=wt, in_=wt_i)
            psstrip = singles.tile([2, HM * W], mybir.dt.float32)
            tmax_strip = singles.tile([128, HM], mybir.dt.float32)
            tidx_strip = singles.tile([128, HM], mybir.dt.uint32)

            if mode.startswith("dma"):
                nc.gpsimd.memset(psstrip, 0.0)
                nc.gpsimd.memset(tmax_strip, 0.0)
                nc.gpsimd.memset(tidx_strip, 0)
            for rep in range(reps):
                _emit_workload(
                    nc, big, small, psump, inp, tgt, wt,
                    psstrip, tmax_strip, tidx_strip, mode=mode,
                )
                # psums last: its producer (ACT copy of the final heatmap)
                # finishes latest, so drain tmax/tidx first
                nc.gpsimd.dma_start(out=tmax_d[:], in_=tmax_strip)
                nc.gpsimd.dma_start(out=tidx_d[:], in_=tidx_strip)
                nc.gpsimd.dma_start(out=psums_d[:], in_=psstrip)

    # walrus path never runs Bacc's pipeline; this splits multi-wait
    # instructions (the final drain) into chained EventSemaphores to meet
    # the 1-wait-per-instruction hardware constraint
    bass_rust.generate_event_semaphores(nc)
    return nc


def _emit_workload(nc, big, small, psump, inp, tgt, wt, psstrip, tmax_strip,
                   tidx_strip, mode="full"):
    if mode == "dmabig":
        TB = 8
        for b in range(BPC):
            in_b = inp[b].rearrange("c (q t) w -> (c q) (t w)", t=TB)
            tgt_b = tgt[b].rearrange("c (q t) w -> (c q) (t w)", t=TB)
            it = big.tile([128, TB * W], mybir.dt.float32, tag="inb")
            nc.sync.dma_start(out=it, in_=in_b)
            tt = big.tile([128, TB * W], mybir.dt.float32, tag="tgtb")
            nc.sync.dma_start(out=tt, in_=tgt_b)
        return
    for m in range(HM):
        b, ch = divmod(m, C)
        in_hm = inp[b, ch].rearrange("(p t) c -> p (t c)", t=RPP)
        tgt_hm = tgt[b, ch].rearrange("(p t) c -> p (t c)", t=RPP)

        # tgt first: the last-finishing DMA then feeds the short ACT/PE tail
        # (exp+matmul ~3.5us) instead of the longer DVE max chain (~4.5us)
        tgt_eng = nc.scalar if mode.endswith("2") else nc.sync
        tgt_tile = big.tile([128, RPP * W], mybir.dt.float32, tag="tgt")
        tgt_eng.dma_start(out=tgt_tile, in_=tgt_hm)
        in_tile = big.tile([128, RPP * W], mybir.dt.float32, tag="in")
        nc.sync.dma_start(out=in_tile, in_=in_hm)

        if mode.startswith("dma"):
            continue

        # in-place exp frees SBUF so the big pool holds all 8 in/tgt tiles
        exp_tile = in_tile
        nc.scalar.activation(
            out=exp_tile,
            in_=in_tile,
            func=mybir.ActivationFunctionType.Exp,
        )

        # PE: [colsum; y-weighted colsum] accumulated over the 4 row blocks
        ps = psump.tile([2, W], mybir.dt.float32)
        for t in range(RPP):
            nc.tensor.matmul(
                out=ps,
                lhsT=wt[:, 2 * t:2 * t + 2],
                rhs=exp_tile[:, t * W:(t + 1) * W],
                start=(t == 0),
                stop=(t == RPP - 1),
            )
        nc.scalar.copy(out=psstrip[:, m * W:(m + 1) * W], in_=ps)

        # target: per-partition top-1 value + flat index, staged into SBUF
        # strips via gpsimd
        tmax8 = small.tile([128, 8], mybir.dt.float32, tag="tmax8")
        tidx8 = small.tile([128, 8], mybir.dt.uint32, tag="tidx8")
        nc.vector.max(out=tmax8, in_=tgt_tile)
        nc.vector.max_index(out=tidx8, in_max=tmax8, in_values=tgt_tile)
        nc.gpsimd.tensor_copy(out=tmax_strip[:, m:m + 1], in_=tmax8[:, 0:1])
        nc.gpsimd.tensor_copy(out=tidx_strip[:, m:m + 1], in_=tidx8[:, 0:1])


def benchmark(input, target, iters=100, warmup=10, reps=1, mode="full"):
    """Steady-state per-call device time via async back-to-back dispatch.
    Returns seconds per call of a NEFF containing `reps` workload copies;
    use the slope over reps to isolate HW kernel time from dispatch."""
    key = (reps, mode)
    if key not in _NC_CACHE:
        _NC_CACHE[key] = _build_bass(reps, mode=mode)
    nc = _NC_CACHE[key]
    import time

    import jax
    from jax.experimental.shard_map import shard_map
    from jax.sharding import Mesh, NamedSharding, PartitionSpec

    from concourse import bass2jax, mybir as _mybir

    bass2jax.install_neuronx_cc_hook()

    part_name = nc.partition_id_tensor.name if nc.partition_id_tensor else None
    in_names, out_names, out_avals, zero_shapes = [], [], [], []
    for alloc in nc.m.functions[0].allocations:
        if not isinstance(alloc, _mybir.MemoryLocationSet):
            continue
        name = alloc.memorylocations[0].name
        if alloc.kind == "ExternalInput":
            if name != part_name:
                in_names.append(name)
        elif alloc.kind == "ExternalOutput":
            out_names.append(name)
            shape = tuple(alloc.tensor_shape)
            dtype = _mybir.dt.np(alloc.dtype)
            out_avals.append(jax.core.ShapedArray(shape, dtype))
            zero_shapes.append((shape, dtype))
    n_params = len(in_names)
    n_outs = len(out_avals)
    all_names = in_names + out_names
    if part_name is not None:
        all_names = all_names + [part_name]
    donate = tuple(range(n_params, n_params + n_outs))

    def _body(*args):
        operands = list(args)
        if part_name is not None:
            operands.append(bass2jax.partition_id_tensor())
        outs = bass2jax._bass_exec_p.bind(
            *operands,
            out_avals=tuple(out_avals),
            in_names=tuple(all_names),
            out_names=tuple(out_names),
            lowering_input_output_aliases=(),
            sim_require_finite=True,
            sim_require_nnan=True,
            nc=nc,
        )
        return tuple(outs)

    devices = jax.devices()[:N_CORES]
    mesh = Mesh(np.asarray(devices), ("core",))
    specs = (PartitionSpec("core"),) * (n_params + n_outs)
    sharded = jax.jit(
        shard_map(
            _body,
            mesh=mesh,
            in_specs=specs,
            out_specs=(PartitionSpec("core"),) * n_outs,
            check_rep=False,
        ),
        donate_argnums=donate,
        keep_unused=True,
    )

    input = np.ascontiguousarray(np.asarray(input, dtype=np.float32))
    target = np.ascontiguousarray(np.asarray(target, dtype=np.float32))
    sh = NamedSharding(mesh, PartitionSpec("core"))
    dev_in = {
        "input": jax.device_put(input.reshape(B, C, H, W), sh),
        "target": jax.device_put(target.reshape(B, C, H, W), sh),
    }
    concat_in = [dev_in[n] for n in in_names]

    # outputs chained device-resident (donated buffers are fully
    # overwritten each call) -- fresh host arrays per call cost ~19ms
    outs = tuple(
        jax.device_put(np.zeros((N_CORES * s[0], *s[1:]), d), sh)
        for s, d in zero_shapes
    )
    for _ in range(warmup):
        outs = sharded(*concat_in, *outs)
    jax.block_until_ready(outs)
    t0 = time.perf_counter()
    for _ in range(iters):
        outs = sharded(*concat_in, *outs)
    jax.block_until_ready(outs)
    t1 = time.perf_counter()
    return (t1 - t0) / iters


def kernel(input, target):
    global LAST_RESULTS
    key = (1, "full")
    if key not in _NC_CACHE:
        _NC_CACHE[key] = _build_bass(1)
    nc = _NC_CACHE[key]

    input = np.ascontiguousarray(np.asarray(input, dtype=np.float32))
    target = np.ascontiguousarray(np.asarray(target, dtype=np.float32))

    in_maps = [
        {
            "input": np.ascontiguousarray(input[i * BPC:(i + 1) * BPC]),
            "target": np.ascontiguousarray(target[i * BPC:(i + 1) * BPC]),
        }
        for i in range(N_CORES)
    ]
    res = run_bass_kernel_spmd(nc, in_maps, list(range(N_CORES)), trace=TRACE)
    LAST_RESULTS = res

    # host epilogue over tiny per-core partials
    p_grid = np.arange(128)
    cgrid = np.arange(W, dtype=np.float64) + 1.0
    ed = np.zeros((B, C), dtype=np.float64)
    for i in range(N_CORES):
        r = res.results[i]
        psums = np.asarray(r["psums"]).astype(np.float64).reshape(2, HM, W)
        tmax = np.asarray(r["tmax"])
        tidx = np.asarray(r["tidx"])
        for m in range(HM):
            b, ch = divmod(m, C)
            gb = i * BPC + b
            cs = psums[0, m]
            yw = psums[1, m]
            s = cs.sum()
            pred_x = (cs * cgrid).sum() / s
            pred_y = yw.sum() / s
            # hard argmax of target (first-occurrence tie-break on flat index)
            vals = tmax[:, m]
            idx = tidx[:, m].astype(np.int64)
            rr = p_grid * RPP + idx // W
            cc = idx % W
            cand = np.flatnonzero(vals == vals.max())
            flat = rr[cand] * W + cc[cand]
            w = cand[np.argmin(flat)]
            true_x = float(cc[w] + 1)
            true_y = float(rr[w] + 1)
            ed[gb, ch] = np.sqrt((true_x - pred_x) ** 2 + (true_y - pred_y) ** 2)

    s_i = ed[:, 0].sum()
    s_s = ed[:, 1:].sum()
    return (
        np.float32(s_i / B),
        np.float32(s_s / B),
        np.float32((s_i + s_s) / B),
    )


# revision 35
# speedup vs baseline: 1.0291x; 1.0291x over previous
import sys

if "/opt/trn_rl_repo" not in sys.path:
    sys.path.insert(0, "/opt/trn_rl_repo")

import numpy as np

import bass_rust
import concourse.bass as bass
import concourse.tile as tile
from concourse import mybir
from concourse.bass_utils import run_bass_kernel_spmd

# Problem constants (nn_PixelDSNTDoubleEval): B,C,H,W = 32,2,512,512 fp32,
# sharded batch-wise across 8 cores -> 4 samples (8 heatmaps) per core.
B, C, H, W = 32, 2, 512, 512
N_CORES = 8
BPC = B // N_CORES          # batches per core
HM = BPC * C                # heatmaps per core
RPP = H // 128              # rows per partition (4)

TRACE = False
LAST_RESULTS = None

_NC_CACHE = {}


def _build_bass(reps=1, mode="full"):
    nc = bass.Bass()
    inp = nc.declare_dram_parameter("input", [BPC, C, H, W], mybir.dt.float32, isOutput=False)
    tgt = nc.declare_dram_parameter("target", [BPC, C, H, W], mybir.dt.float32, isOutput=False)
    # psums row 0 = column sums of exp; row 1 = (row+1)-weighted column sums
    # last heatmap is processed in RPP column blocks -> RPP strip entries
    psums_d = nc.declare_dram_parameter("psums", [2, HM * W], mybir.dt.float32, isOutput=True)
    tmax_d = nc.declare_dram_parameter("tmax", [128, HM - 1 + RPP], mybir.dt.float32, isOutput=True)
    tidx_d = nc.declare_dram_parameter("tidx", [128, HM - 1 + RPP], mybir.dt.uint32, isOutput=True)

    with tile.TileContext(nc) as tc:
        with (
            tc.tile_pool(name="big", bufs=(4 if mode == "dmabig" else HM)) as big,
            tc.tile_pool(name="small", bufs=HM) as small,
            tc.tile_pool(name="psum", bufs=2, space="PSUM") as psump,
            tc.tile_pool(name="singles", bufs=1) as singles,
        ):
            # stationary weights: per t-block a (128,2) pair [1, 4p+t+1].
            # Built entirely on gpsimd, then relayed through one ACT copy so
            # matmuls only ever wait on the ACT semaphore (TPB instructions
            # support a single sync-wait slot).
            wt_i = singles.tile([128, 2 * RPP], mybir.dt.float32)
            nc.gpsimd.memset(wt_i, 1.0)
            for t in range(RPP):
                nc.gpsimd.iota(
                    wt_i[:, 2 * t + 1:2 * t + 2],
                    pattern=[[1, 1]],
                    base=t + 1,
                    channel_multiplier=RPP,
                    allow_small_or_imprecise_dtypes=True,
                )
            wt = singles.tile([128, 2 * RPP], mybir.dt.float32)
            nc.scalar.copy(out=wt, in_=wt_i)
            psstrip = singles.tile([2, HM * W], mybir.dt.float32)
            tmax_strip = singles.tile([128, HM - 1 + RPP], mybir.dt.float32)
            tidx_strip = singles.tile([128, HM - 1 + RPP], mybir.dt.uint32)

            if mode.startswith("dma"):
                nc.gpsimd.memset(psstrip, 0.0)
                nc.gpsimd.memset(tmax_strip, 0.0)
                nc.gpsimd.memset(tidx_strip, 0)
            for rep in range(reps):
                _emit_workload(
                    nc, big, small, psump, inp, tgt, wt,
                    psstrip, tmax_strip, tidx_strip, mode=mode,
                )
                # psums last: its producer (ACT copy of the final heatmap)
                # finishes latest, so drain tmax/tidx first
                nc.gpsimd.dma_start(out=tmax_d[:], in_=tmax_strip)
                nc.gpsimd.dma_start(out=tidx_d[:], in_=tidx_strip)
                nc.gpsimd.dma_start(out=psums_d[:], in_=psstrip)

    # walrus path never runs Bacc's pipeline; this splits multi-wait
    # instructions (the final drain) into chained EventSemaphores to meet
    # the 1-wait-per-instruction hardware constraint
    bass_rust.generate_event_semaphores(nc)
    return nc


def _emit_workload(nc, big, small, psump, inp, tgt, wt, psstrip, tmax_strip,
                   tidx_strip, mode="full"):
    if mode == "dmabig":
        TB = 8
        for b in range(BPC):
            in_b = inp[b].rearrange("c (q t) w -> (c q) (t w)", t=TB)
            tgt_b = tgt[b].rearrange("c (q t) w -> (c q) (t w)", t=TB)
            it = big.tile([128, TB * W], mybir.dt.float32, tag="inb")
            nc.sync.dma_start(out=it, in_=in_b)
            tt = big.tile([128, TB * W], mybir.dt.float32, tag="tgtb")
            nc.sync.dma_start(out=tt, in_=tgt_b)
        return
    n_whole = HM if mode.startswith("dma") else HM - 1
    for m in range(n_whole):
        b, ch = divmod(m, C)
        in_hm = inp[b, ch].rearrange("(p t) c -> p (t c)", t=RPP)
        tgt_hm = tgt[b, ch].rearrange("(p t) c -> p (t c)", t=RPP)

        # tgt first: the last-finishing DMA then feeds the short ACT/PE tail
        # (exp+matmul ~3.5us) instead of the longer DVE max chain (~4.5us)
        tgt_eng = nc.scalar if mode.endswith("2") else nc.sync
        tgt_tile = big.tile([128, RPP * W], mybir.dt.float32, tag="tgt")
        tgt_eng.dma_start(out=tgt_tile, in_=tgt_hm)
        in_tile = big.tile([128, RPP * W], mybir.dt.float32, tag="in")
        nc.sync.dma_start(out=in_tile, in_=in_hm)

        if mode.startswith("dma"):
            continue

        # in-place exp frees SBUF so the big pool holds all 8 in/tgt tiles
        exp_tile = in_tile
        nc.scalar.activation(
            out=exp_tile,
            in_=in_tile,
            func=mybir.ActivationFunctionType.Exp,
        )

        # PE: [colsum; y-weighted colsum] accumulated over the 4 row blocks
        ps = psump.tile([2, W], mybir.dt.float32)
        for t in range(RPP):
            nc.tensor.matmul(
                out=ps,
                lhsT=wt[:, 2 * t:2 * t + 2],
                rhs=exp_tile[:, t * W:(t + 1) * W],
                start=(t == 0),
                stop=(t == RPP - 1),
            )
        nc.scalar.copy(out=psstrip[:, m * W:(m + 1) * W], in_=ps)

        # target: per-partition top-1 value + flat index, staged into SBUF
        # strips via gpsimd
        tmax8 = small.tile([128, 8], mybir.dt.float32, tag="tmax8")
        tidx8 = small.tile([128, 8], mybir.dt.uint32, tag="tidx8")
        nc.vector.max(out=tmax8, in_=tgt_tile)
        nc.vector.max_index(out=tidx8, in_max=tmax8, in_values=tgt_tile)
        nc.gpsimd.tensor_copy(out=tmax_strip[:, m:m + 1], in_=tmax8[:, 0:1])
        nc.gpsimd.tensor_copy(out=tidx_strip[:, m:m + 1], in_=tidx8[:, 0:1])

    if mode.startswith("dma"):
        return

    # last heatmap split into RPP 512-col blocks: compute starts per block
    # as it lands, shrinking the single-shot tail after the final DMA
    m = HM - 1
    b, ch = divmod(m, C)
    in_hm = inp[b, ch].rearrange("(p t) c -> p (t c)", t=RPP)
    tgt_hm = tgt[b, ch].rearrange("(p t) c -> p (t c)", t=RPP)
    ps = psump.tile([2, W], mybir.dt.float32)
    for t in range(RPP):
        tgt_blk = big.tile([128, W], mybir.dt.float32, tag="tgtlast")
        nc.sync.dma_start(out=tgt_blk, in_=tgt_hm[:, t * W:(t + 1) * W])
        in_blk = big.tile([128, W], mybir.dt.float32, tag="inlast")
        nc.sync.dma_start(out=in_blk, in_=in_hm[:, t * W:(t + 1) * W])

        nc.scalar.activation(
            out=in_blk, in_=in_blk, func=mybir.ActivationFunctionType.Exp,
        )
        nc.tensor.matmul(
            out=ps,
            lhsT=wt[:, 2 * t:2 * t + 2],
            rhs=in_blk,
            start=(t == 0),
            stop=(t == RPP - 1),
        )

        tmax8 = small.tile([128, 8], mybir.dt.float32, tag="tmax8")
        tidx8 = small.tile([128, 8], mybir.dt.uint32, tag="tidx8")
        nc.vector.max(out=tmax8, in_=tgt_blk)
        nc.vector.max_index(out=tidx8, in_max=tmax8, in_values=tgt_blk)
        nc.gpsimd.tensor_copy(out=tmax_strip[:, m + t:m + t + 1], in_=tmax8[:, 0:1])
        nc.gpsimd.tensor_copy(out=tidx_strip[:, m + t:m + t + 1], in_=tidx8[:, 0:1])
    nc.scalar.copy(out=psstrip[:, m * W:(m + 1) * W], in_=ps)


def benchmark(input, target, iters=100, warmup=10, reps=1, mode="full"):
    """Steady-state per-call device time via async back-to-back dispatch.
    Returns seconds per call of a NEFF containing `reps` workload copies;
    use the slope over reps to isolate HW kernel time from dispatch."""
    key = (reps, mode)
    if key not in _NC_CACHE:
        _NC_CACHE[key] = _build_bass(reps, mode=mode)
    nc = _NC_CACHE[key]
    import time

    import jax
    from jax.experimental.shard_map import shard_map
    from jax.sharding import Mesh, NamedSharding, PartitionSpec

    from concourse import bass2jax, mybir as _mybir

    bass2jax.install_neuronx_cc_hook()

    part_name = nc.partition_id_tensor.name if nc.partition_id_tensor else None
    in_names, out_names, out_avals, zero_shapes = [], [], [], []
    for alloc in nc.m.functions[0].allocations:
        if not isinstance(alloc, _mybir.MemoryLocationSet):
            continue
        name = alloc.memorylocations[0].name
        if alloc.kind == "ExternalInput":
            if name != part_name:
                in_names.append(name)
        elif alloc.kind == "ExternalOutput":
            out_names.append(name)
            shape = tuple(alloc.tensor_shape)
            dtype = _mybir.dt.np(alloc.dtype)
            out_avals.append(jax.core.ShapedArray(shape, dtype))
            zero_shapes.append((shape, dtype))
    n_params = len(in_names)
    n_outs = len(out_avals)
    all_names = in_names + out_names
    if part_name is not None:
        all_names = all_names + [part_name]
    donate = tuple(range(n_params, n_params + n_outs))

    def _body(*args):
        operands = list(args)
        if part_name is not None:
            operands.append(bass2jax.partition_id_tensor())
        outs = bass2jax._bass_exec_p.bind(
            *operands,
            out_avals=tuple(out_avals),
            in_names=tuple(all_names),
            out_names=tuple(out_names),
            lowering_input_output_aliases=(),
            sim_require_finite=True,
            sim_require_nnan=True,
            nc=nc,
        )
        return tuple(outs)

    devices = jax.devices()[:N_CORES]
    mesh = Mesh(np.asarray(devices), ("core",))
    specs = (PartitionSpec("core"),) * (n_params + n_outs)
    sharded = jax.jit(
        shard_map(
            _body,
            mesh=mesh,
            in_specs=specs,
            out_specs=(PartitionSpec("core"),) * n_outs,
            check_rep=False,
        ),
        donate_argnums=donate,
        keep_unused=True,
    )

    input = np.ascontiguousarray(np.asarray(input, dtype=np.float32))
    target = np.ascontiguousarray(np.asarray(target, dtype=np.float32))
    sh = NamedSharding(mesh, PartitionSpec("core"))
    dev_in = {
        "input": jax.device_put(input.reshape(B, C, H, W), sh),
        "target": jax.device_put(target.reshape(B, C, H, W), sh),
    }
    concat_in = [dev_in[n] for n in in_names]

    # outputs chained device-resident (donated buffers are fully
    # overwritten each call) -- fresh host arrays per call cost ~19ms
    outs = tuple(
        jax.device_put(np.zeros((N_CORES * s[0], *s[1:]), d), sh)
        for s, d in zero_shapes
    )
    for _ in range(warmup):
        outs = sharded(*concat_in, *outs)
    jax.block_until_ready(outs)
    t0 = time.perf_counter()
    for _ in range(iters):
        outs = sharded(*concat_in, *outs)
    jax.block_until_ready(outs)
    t1 = time.perf_counter()
    return (t1 - t0) / iters


def kernel(input, target):
    global LAST_RESULTS
    key = (1, "full")
    if key not in _NC_CACHE:
        _NC_CACHE[key] = _build_bass(1)
    nc = _NC_CACHE[key]

    input = np.ascontiguousarray(np.asarray(input, dtype=np.float32))
    target = np.ascontiguousarray(np.asarray(target, dtype=np.float32))

    in_maps = [
        {
            "input": np.ascontiguousarray(input[i * BPC:(i + 1) * BPC]),
            "target": np.ascontiguousarray(target[i * BPC:(i + 1) * BPC]),
        }
        for i in range(N_CORES)
    ]
    res = run_bass_kernel_spmd(nc, in_maps, list(range(N_CORES)), trace=TRACE)
    LAST_RESULTS = res

    # host epilogue over tiny per-core partials
    p_grid = np.arange(128)
    cgrid = np.arange(W, dtype=np.float64) + 1.0
    ed = np.zeros((B, C), dtype=np.float64)
    for i in range(N_CORES):
        r = res.results[i]
        psums = np.asarray(r["psums"]).astype(np.float64).reshape(2, HM, W)
        tmax = np.asarray(r["tmax"])
        tidx = np.asarray(r["tidx"])
        for m in range(HM):
            b, ch = divmod(m, C)
            gb = i * BPC + b
            cs = psums[0, m]
            yw = psums[1, m]
            s = cs.sum()
            pred_x = (cs * cgrid).sum() / s
            pred_y = yw.sum() / s
            # hard argmax of target (first-occurrence tie-break on flat index)
            if m < HM - 1:
                vals = tmax[:, m]
                idx = tidx[:, m].astype(np.int64)
                rows = p_grid * RPP + idx // W
                cols = idx % W
            else:
                # last heatmap was processed in RPP blocks: strip cols
                # m..m+RPP-1, block t holds image rows {RPP*p + t}
                vals = tmax[:, m:m + RPP].ravel()
                cols = tidx[:, m:m + RPP].astype(np.int64).ravel()
                rows = (p_grid[:, None] * RPP + np.arange(RPP)[None, :]).ravel()
            flat = rows * W + cols
            cand = np.flatnonzero(vals == vals.max())
            w = cand[np.argmin(flat[cand])]
            true_x = float(cols[w] + 1)
            true_y = float(rows[w] + 1)
            ed[gb, ch] = np.sqrt((true_x - pred_x) ** 2 + (true_y - pred_y) ** 2)

    s_i = ed[:, 0].sum()
    s_s = ed[:, 1:].sum()
    return (
        np.float32(s_i / B),
        np.float32(s_s / B),
        np.float32((s_i + s_s) / B),
    )


# revision 40
# speedup vs baseline: 1.0389x; 1.0095x over previous
import sys

if "/opt/trn_rl_repo" not in sys.path:
    sys.path.insert(0, "/opt/trn_rl_repo")

import numpy as np

import bass_rust
import concourse.bass as bass
import concourse.tile as tile
from concourse import mybir
from concourse.bass_utils import run_bass_kernel_spmd

# Problem constants (nn_PixelDSNTDoubleEval): B,C,H,W = 32,2,512,512 fp32,
# sharded batch-wise across 8 cores -> 4 samples (8 heatmaps) per core.
B, C, H, W = 32, 2, 512, 512
N_CORES = 8
BPC = B // N_CORES          # batches per core
HM = BPC * C                # heatmaps per core
RPP = H // 128              # rows per partition (4)

TRACE = False
LAST_RESULTS = None

_NC_CACHE = {}


def _build_bass(reps=1, mode="full"):
    nc = bass.Bass()
    inp = nc.declare_dram_parameter("input", [BPC, C, H, W], mybir.dt.float32, isOutput=False)
    tgt = nc.declare_dram_parameter("target", [BPC, C, H, W], mybir.dt.float32, isOutput=False)
    # psums row 0 = column sums of exp; row 1 = (row+1)-weighted column sums
    psums_d = nc.declare_dram_parameter("psums", [2, HM * W], mybir.dt.float32, isOutput=True)
    tmax_d = nc.declare_dram_parameter("tmax", [128, HM], mybir.dt.float32, isOutput=True)
    tidx_d = nc.declare_dram_parameter("tidx", [128, HM], mybir.dt.uint32, isOutput=True)

    with tile.TileContext(nc) as tc:
        with (
            tc.tile_pool(name="big", bufs=(4 if mode == "dmabig" else HM)) as big,
            tc.tile_pool(name="small", bufs=HM) as small,
            tc.tile_pool(name="psum", bufs=2, space="PSUM") as psump,
            tc.tile_pool(name="singles", bufs=1) as singles,
        ):
            # stationary weights: per t-block a (128,2) pair [1, 4p+t+1].
            # Built entirely on gpsimd, then relayed through one ACT copy so
            # matmuls only ever wait on the ACT semaphore (TPB instructions
            # support a single sync-wait slot).
            wt_i = singles.tile([128, 2 * RPP], mybir.dt.float32)
            nc.gpsimd.memset(wt_i, 1.0)
            for t in range(RPP):
                nc.gpsimd.iota(
                    wt_i[:, 2 * t + 1:2 * t + 2],
                    pattern=[[1, 1]],
                    base=t + 1,
                    channel_multiplier=RPP,
                    allow_small_or_imprecise_dtypes=True,
                )
            wt = singles.tile([128, 2 * RPP], mybir.dt.float32)
            nc.scalar.copy(out=wt, in_=wt_i)
            psstrip = singles.tile([2, HM * W], mybir.dt.float32)
            tmax_strip = singles.tile([128, HM], mybir.dt.float32)
            tidx_strip = singles.tile([128, HM], mybir.dt.uint32)

            if mode.startswith("dma"):
                nc.gpsimd.memset(psstrip, 0.0)
                nc.gpsimd.memset(tmax_strip, 0.0)
                nc.gpsimd.memset(tidx_strip, 0)
            for rep in range(reps):
                _emit_workload(
                    nc, big, small, psump, inp, tgt, wt,
                    psstrip, tmax_strip, tidx_strip, mode=mode,
                )
                # psums last: its producer (ACT copy of the final heatmap)
                # finishes latest, so drain tmax/tidx first
                nc.gpsimd.dma_start(out=tmax_d[:], in_=tmax_strip)
                nc.gpsimd.dma_start(out=tidx_d[:], in_=tidx_strip)
                nc.gpsimd.dma_start(out=psums_d[:], in_=psstrip)

    # walrus path never runs Bacc's pipeline; this splits multi-wait
    # instructions (the final drain) into chained EventSemaphores to meet
    # the 1-wait-per-instruction hardware constraint
    bass_rust.generate_event_semaphores(nc)
    return nc


def _emit_workload(nc, big, small, psump, inp, tgt, wt, psstrip, tmax_strip,
                   tidx_strip, mode="full"):
    if mode == "dmabig":
        TB = 8
        for b in range(BPC):
            in_b = inp[b].rearrange("c (q t) w -> (c q) (t w)", t=TB)
            tgt_b = tgt[b].rearrange("c (q t) w -> (c q) (t w)", t=TB)
            it = big.tile([128, TB * W], mybir.dt.float32, tag="inb")
            nc.sync.dma_start(out=it, in_=in_b)
            tt = big.tile([128, TB * W], mybir.dt.float32, tag="tgtb")
            nc.sync.dma_start(out=tt, in_=tgt_b)
        return
    for m in range(HM):
        b, ch = divmod(m, C)
        in_hm = inp[b, ch].rearrange("(p t) c -> p (t c)", t=RPP)
        tgt_hm = tgt[b, ch].rearrange("(p t) c -> p (t c)", t=RPP)

        # tgt first: the last-finishing DMA then feeds the short ACT/PE tail
        # (exp+matmul ~3.5us) instead of the longer DVE max chain (~4.5us)
        tgt_eng = nc.scalar if mode.endswith("2") else nc.sync
        tgt_tile = big.tile([128, RPP * W], mybir.dt.float32, tag="tgt")
        tgt_eng.dma_start(out=tgt_tile, in_=tgt_hm)
        in_tile = big.tile([128, RPP * W], mybir.dt.float32, tag="in")
        nc.sync.dma_start(out=in_tile, in_=in_hm)

        if mode.startswith("dma"):
            continue

        # in-place exp frees SBUF so the big pool holds all 8 in/tgt tiles
        exp_tile = in_tile
        nc.scalar.activation(
            out=exp_tile,
            in_=in_tile,
            func=mybir.ActivationFunctionType.Exp,
        )

        # PE: [colsum; y-weighted colsum] accumulated over the 4 row blocks
        ps = psump.tile([2, W], mybir.dt.float32)
        for t in range(RPP):
            nc.tensor.matmul(
                out=ps,
                lhsT=wt[:, 2 * t:2 * t + 2],
                rhs=exp_tile[:, t * W:(t + 1) * W],
                start=(t == 0),
                stop=(t == RPP - 1),
            )
        nc.scalar.copy(out=psstrip[:, m * W:(m + 1) * W], in_=ps)

        # target: per-partition top-1 value + flat index, staged into SBUF
        # strips via gpsimd
        tmax8 = small.tile([128, 8], mybir.dt.float32, tag="tmax8")
        tidx8 = small.tile([128, 8], mybir.dt.uint32, tag="tidx8")
        nc.vector.max(out=tmax8, in_=tgt_tile)
        nc.vector.max_index(out=tidx8, in_max=tmax8, in_values=tgt_tile)
        nc.gpsimd.tensor_copy(out=tmax_strip[:, m:m + 1], in_=tmax8[:, 0:1])
        nc.gpsimd.tensor_copy(out=tidx_strip[:, m:m + 1], in_=tidx8[:, 0:1])


def benchmark(input, target, iters=100, warmup=10, reps=1, mode="full"):
    """Steady-state per-call device time via async back-to-back dispatch.
    Returns seconds per call of a NEFF containing `reps` workload copies;
    use the slope over reps to isolate HW kernel time from dispatch."""
    key = (reps, mode)
    if key not in _NC_CACHE:
        _NC_CACHE[key] = _build_bass(reps, mode=mode)
    nc = _NC_CACHE[key]
    import time

    import jax
    from jax.experimental.shard_map import shard_map
    from jax.sharding import Mesh, NamedSharding, PartitionSpec

    from concourse import bass2jax, mybir as _mybir

    bass2jax.install_neuronx_cc_hook()

    part_name = nc.partition_id_tensor.name if nc.partition_id_tensor else None
    in_names, out_names, out_avals, zero_shapes = [], [], [], []
    for alloc in nc.m.functions[0].allocations:
        if not isinstance(alloc, _mybir.MemoryLocationSet):
            continue
        name = alloc.memorylocations[0].name
        if alloc.kind == "ExternalInput":
            if name != part_name:
                in_names.append(name)
        elif alloc.kind == "ExternalOutput":
            out_names.append(name)
            shape = tuple(alloc.tensor_shape)
            dtype = _mybir.dt.np(alloc.dtype)
            out_avals.append(jax.core.ShapedArray(shape, dtype))
            zero_shapes.append((shape, dtype))
    n_params = len(in_names)
    n_outs = len(out_avals)
    all_names = in_names + out_names
    if part_name is not None:
        all_names = all_names + [part_name]
    donate = tuple(range(n_params, n_params + n_outs))

    def _body(*args):
        operands = list(args)
        if part_name is not None:
            operands.append(bass2jax.partition_id_tensor())
        outs = bass2jax._bass_exec_p.bind(
            *operands,
            out_avals=tuple(out_avals),
            in_names=tuple(all_names),
            out_names=tuple(out_names),
            lowering_input_output_aliases=(),
            sim_require_finite=True,
            sim_require_nnan=True,
            nc=nc,
        )
        return tuple(outs)

    devices = jax.devices()[:N_CORES]
    mesh = Mesh(np.asarray(devices), ("core",))
    specs = (PartitionSpec("core"),) * (n_params + n_outs)
    sharded = jax.jit(
        shard_map(
            _body,
            mesh=mesh,
            in_specs=specs,
            out_specs=(PartitionSpec("core"),) * n_outs,
            check_rep=False,
        ),
        donate_argnums=donate,
        keep_unused=True,
    )

    input = np.ascontiguousarray(np.asarray(input, dtype=np.float32))
    target = np.ascontiguousarray(np.asarray(target, dtype=np.float32))
    sh = NamedSharding(mesh, PartitionSpec("core"))
    dev_in = {
        "input": jax.device_put(input.reshape(B, C, H, W), sh),
        "target": jax.device_put(target.reshape(B, C, H, W), sh),
    }
    concat_in = [dev_in[n] for n in in_names]

    # outputs chained device-resident (donated buffers are fully
    # overwritten each call) -- fresh host arrays per call cost ~19ms
    outs = tuple(
        jax.device_put(np.zeros((N_CORES * s[0], *s[1:]), d), sh)
        for s, d in zero_shapes
    )
    for _ in range(warmup):
        outs = sharded(*concat_in, *outs)
    jax.block_until_ready(outs)
    t0 = time.perf_counter()
    for _ in range(iters):
        outs = sharded(*concat_in, *outs)
    jax.block_until_ready(outs)
    t1 = time.perf_counter()
    return (t1 - t0) / iters


def kernel(input, target):
    global LAST_RESULTS
    key = (1, "full")
    if key not in _NC_CACHE:
        _NC_CACHE[key] = _build_bass(1)
    nc = _NC_CACHE[key]

    input = np.ascontiguousarray(np.asarray(input, dtype=np.float32))
    target = np.ascontiguousarray(np.asarray(target, dtype=np.float32))

    in_maps = [
        {
            "input": np.ascontiguousarray(input[i * BPC:(i + 1) * BPC]),
            "target": np.ascontiguousarray(target[i * BPC:(i + 1) * BPC]),
        }
        for i in range(N_CORES)
    ]
    res = run_bass_kernel_spmd(nc, in_maps, list(range(N_CORES)), trace=TRACE)
    LAST_RESULTS = res

    # host epilogue over tiny per-core partials
    p_grid = np.arange(128)
    cgrid = np.arange(W, dtype=np.float64) + 1.0
    ed = np.zeros((B, C), dtype=np.float64)
    for i in range(N_CORES):
        r = res.results[i]
        psums = np.asarray(r["psums"]).astype(np.float64).reshape(2, HM, W)
        tmax = np.asarray(r["tmax"])
        tidx = np.asarray(r["tidx"])
        for m in range(HM):
            b, ch = divmod(m, C)
            gb = i * BPC + b
            cs = psums[0, m]
            yw = psums[1, m]
            s = cs.sum()
            pred_x = (cs * cgrid).sum() / s
            pred_y = yw.sum() / s
            # hard argmax of target (first-occurrence tie-break on flat index)
            vals = tmax[:, m]
            idx = tidx[:, m].astype(np.int64)
            rows = p_grid * RPP + idx // W
            cols = idx % W
            flat = rows * W + cols
            cand = np.flatnonzero(vals == vals.max())
            w = cand[np.argmin(flat[cand])]
            true_x = float(cols[w] + 1)
            true_y = float(rows[w] + 1)
            ed[gb, ch] = np.sqrt((true_x - pred_x) ** 2 + (true_y - pred_y) ** 2)

    s_i = ed[:, 0].sum()
    s_s = ed[:, 1:].sum()
    return (
        np.float32(s_i / B),
        np.float32(s_s / B),
        np.float32((s_i + s_s) / B),
    )


# revision 41
# speedup vs baseline: 31.0258x; 29.8654x over previous
import sys

if "/opt/trn_rl_repo" not in sys.path:
    sys.path.insert(0, "/opt/trn_rl_repo")

import numpy as np

import bass_rust
import concourse.bass as bass
import concourse.tile as tile
from concourse import mybir
from concourse.bass_utils import run_bass_kernel_spmd

# Problem constants (nn_PixelDSNTDoubleEval): B,C,H,W = 32,2,512,512 fp32,
# sharded batch-wise across 8 cores -> 4 samples (8 heatmaps) per core.
B, C, H, W = 32, 2, 512, 512
N_CORES = 8
BPC = B // N_CORES          # batches per core
HM = BPC * C                # heatmaps per core
RPP = H // 128              # rows per partition (4)

TRACE = False
LAST_RESULTS = None

_NC_CACHE = {}


def _build_bass(reps=1, mode="full"):
    nc = bass.Bass()
    inp = nc.declare_dram_parameter("input", [BPC, C, H, W], mybir.dt.float32, isOutput=False)
    tgt = nc.declare_dram_parameter("target", [BPC, C, H, W], mybir.dt.float32, isOutput=False)
    # psums row 0 = column sums of exp; row 1 = (row+1)-weighted column sums
    psums_d = nc.declare_dram_parameter("psums", [2, HM * W], mybir.dt.float32, isOutput=True)
    tmax_d = nc.declare_dram_parameter("tmax", [128, HM], mybir.dt.float32, isOutput=True)
    tidx_d = nc.declare_dram_parameter("tidx", [128, HM], mybir.dt.uint32, isOutput=True)

    with tile.TileContext(nc) as tc:
        with (
            tc.tile_pool(name="big", bufs=(4 if mode == "dmabig" else HM)) as big,
            tc.tile_pool(name="small", bufs=HM) as small,
            tc.tile_pool(name="psum", bufs=2, space="PSUM") as psump,
            tc.tile_pool(name="singles", bufs=1) as singles,
        ):
            # stationary weights: per t-block a (128,2) pair [1, 4p+t+1].
            # Built entirely on gpsimd, then relayed through one ACT copy so
            # matmuls only ever wait on the ACT semaphore (TPB instructions
            # support a single sync-wait slot).
            wt_i = singles.tile([128, 2 * RPP], mybir.dt.float32)
            nc.gpsimd.memset(wt_i, 1.0)
            for t in range(RPP):
                nc.gpsimd.iota(
                    wt_i[:, 2 * t + 1:2 * t + 2],
                    pattern=[[1, 1]],
                    base=t + 1,
                    channel_multiplier=RPP,
                    allow_small_or_imprecise_dtypes=True,
                )
            wt = singles.tile([128, 2 * RPP], mybir.dt.float32)
            nc.scalar.copy(out=wt, in_=wt_i)
            psstrip = singles.tile([2, HM * W], mybir.dt.float32)
            tmax_strip = singles.tile([128, HM], mybir.dt.float32)
            tidx_strip = singles.tile([128, HM], mybir.dt.uint32)

            if mode.startswith("dma"):
                nc.gpsimd.memset(psstrip, 0.0)
                nc.gpsimd.memset(tmax_strip, 0.0)
                nc.gpsimd.memset(tidx_strip, 0)
            for rep in range(reps):
                _emit_workload(
                    nc, big, small, psump, inp, tgt, wt,
                    psstrip, tmax_strip, tidx_strip, mode=mode,
                )
                # psums last: its producer (ACT copy of the final heatmap)
                # finishes latest, so drain tmax/tidx first
                nc.gpsimd.dma_start(out=tmax_d[:], in_=tmax_strip)
                nc.gpsimd.dma_start(out=tidx_d[:], in_=tidx_strip)
                nc.gpsimd.dma_start(out=psums_d[:], in_=psstrip)

    # walrus path never runs Bacc's pipeline; this splits multi-wait
    # instructions (the final drain) into chained EventSemaphores to meet
    # the 1-wait-per-instruction hardware constraint
    bass_rust.generate_event_semaphores(nc)
    return nc


def _emit_workload(nc, big, small, psump, inp, tgt, wt, psstrip, tmax_strip,
                   tidx_strip, mode="full"):
    if mode == "dmabig":
        TB = 8
        for b in range(BPC):
            in_b = inp[b].rearrange("c (q t) w -> (c q) (t w)", t=TB)
            tgt_b = tgt[b].rearrange("c (q t) w -> (c q) (t w)", t=TB)
            it = big.tile([128, TB * W], mybir.dt.float32, tag="inb")
            nc.sync.dma_start(out=it, in_=in_b)
            tt = big.tile([128, TB * W], mybir.dt.float32, tag="tgtb")
            nc.sync.dma_start(out=tt, in_=tgt_b)
        return
    for m in range(HM):
        b, ch = divmod(m, C)
        in_hm = inp[b, ch].rearrange("(p t) c -> p (t c)", t=RPP)
        tgt_hm = tgt[b, ch].rearrange("(p t) c -> p (t c)", t=RPP)

        # tgt first: the last-finishing DMA then feeds the short ACT/PE tail
        # (exp+matmul ~3.5us) instead of the longer DVE max chain (~4.5us)
        tgt_eng = nc.scalar if mode.endswith("2") else nc.sync
        if mode.endswith("A"):
            in_tile = big.tile([128, RPP * W], mybir.dt.float32, tag="in")
            nc.sync.dma_start(out=in_tile, in_=in_hm)
            tgt_tile = big.tile([128, RPP * W], mybir.dt.float32, tag="tgt")
            tgt_eng.dma_start(out=tgt_tile, in_=tgt_hm)
        else:
            tgt_tile = big.tile([128, RPP * W], mybir.dt.float32, tag="tgt")
            tgt_eng.dma_start(out=tgt_tile, in_=tgt_hm)
            in_tile = big.tile([128, RPP * W], mybir.dt.float32, tag="in")
            nc.sync.dma_start(out=in_tile, in_=in_hm)

        if mode.startswith("dma"):
            continue

        # in-place exp frees SBUF so the big pool holds all 8 in/tgt tiles
        exp_tile = in_tile
        nc.scalar.activation(
            out=exp_tile,
            in_=in_tile,
            func=mybir.ActivationFunctionType.Exp,
        )

        # PE: [colsum; y-weighted colsum] accumulated over the 4 row blocks
        ps = psump.tile([2, W], mybir.dt.float32)
        for t in range(RPP):
            nc.tensor.matmul(
                out=ps,
                lhsT=wt[:, 2 * t:2 * t + 2],
                rhs=exp_tile[:, t * W:(t + 1) * W],
                start=(t == 0),
                stop=(t == RPP - 1),
            )
        nc.scalar.copy(out=psstrip[:, m * W:(m + 1) * W], in_=ps)

        # target: per-partition top-1 value + flat index, staged into SBUF
        # strips via gpsimd
        tmax8 = small.tile([128, 8], mybir.dt.float32, tag="tmax8")
        tidx8 = small.tile([128, 8], mybir.dt.uint32, tag="tidx8")
        nc.vector.max(out=tmax8, in_=tgt_tile)
        nc.vector.max_index(out=tidx8, in_max=tmax8, in_values=tgt_tile)
        nc.gpsimd.tensor_copy(out=tmax_strip[:, m:m + 1], in_=tmax8[:, 0:1])
        nc.gpsimd.tensor_copy(out=tidx_strip[:, m:m + 1], in_=tidx8[:, 0:1])


def benchmark(input, target, iters=100, warmup=10, reps=1, mode="full"):
    """Steady-state per-call device time via async back-to-back dispatch.
    Returns seconds per call of a NEFF containing `reps` workload copies;
    use the slope over reps to isolate HW kernel time from dispatch."""
    key = (reps, mode)
    if key not in _NC_CACHE:
        _NC_CACHE[key] = _build_bass(reps, mode=mode)
    nc = _NC_CACHE[key]
    import time

    import jax
    from jax.experimental.shard_map import shard_map
    from jax.sharding import Mesh, NamedSharding, PartitionSpec

    from concourse import bass2jax, mybir as _mybir

    bass2jax.install_neuronx_cc_hook()

    part_name = nc.partition_id_tensor.name if nc.partition_id_tensor else None
    in_names, out_names, out_avals, zero_shapes = [], [], [], []
    for alloc in nc.m.functions[0].allocations:
        if not isinstance(alloc, _mybir.MemoryLocationSet):
            continue
        name = alloc.memorylocations[0].name
        if alloc.kind == "ExternalInput":
            if name != part_name:
                in_names.append(name)
        elif alloc.kind == "ExternalOutput":
            out_names.append(name)
            shape = tuple(alloc.tensor_shape)
            dtype = _mybir.dt.np(alloc.dtype)
            out_avals.append(jax.core.ShapedArray(shape, dtype))
            zero_shapes.append((shape, dtype))
    n_params = len(in_names)
    n_outs = len(out_avals)
    all_names = in_names + out_names
    if part_name is not None:
        all_names = all_names + [part_name]
    donate = tuple(range(n_params, n_params + n_outs))

    def _body(*args):
        operands = list(args)
        if part_name is not None:
            operands.append(bass2jax.partition_id_tensor())
        outs = bass2jax._bass_exec_p.bind(
            *operands,
            out_avals=tuple(out_avals),
            in_names=tuple(all_names),
            out_names=tuple(out_names),
            lowering_input_output_aliases=(),
            sim_require_finite=True,
            sim_require_nnan=True,
            nc=nc,
        )
        return tuple(outs)

    devices = jax.devices()[:N_CORES]
    mesh = Mesh(np.asarray(devices), ("core",))
    specs = (PartitionSpec("core"),) * (n_params + n_outs)
    sharded = jax.jit(
        shard_map(
            _body,
            mesh=mesh,
            in_specs=specs,
            out_specs=(PartitionSpec("core"),) * n_outs,
            check_rep=False,
        ),
        donate_argnums=donate,
        keep_unused=True,
    )

    input = np.ascontiguousarray(np.asarray(input, dtype=np.float32))
    target = np.ascontiguousarray(np.asarray(target, dtype=np.float32))
    sh = NamedSharding(mesh, PartitionSpec("core"))
    dev_in = {
        "input": jax.device_put(input.reshape(B, C, H, W), sh),
        "target": jax.device_put(target.reshape(B, C, H, W), sh),
    }
    concat_in = [dev_in[n] for n in in_names]

    # outputs chained device-resident (donated buffers are fully
    # overwritten each call) -- fresh host arrays per call cost ~19ms
    outs = tuple(
        jax.device_put(np.zeros((N_CORES * s[0], *s[1:]), d), sh)
        for s, d in zero_shapes
    )
    for _ in range(warmup):
        outs = sharded(*concat_in, *outs)
    jax.block_until_ready(outs)
    t0 = time.perf_counter()
    for _ in range(iters):
        outs = sharded(*concat_in, *outs)
    jax.block_until_ready(outs)
    t1 = time.perf_counter()
    return (t1 - t0) / iters


def kernel(input, target):
    global LAST_RESULTS
    key = (1, "full")
    if key not in _NC_CACHE:
        _NC_CACHE[key] = _build_bass(1)
    nc = _NC_CACHE[key]

    input = np.ascontiguousarray(np.asarray(input, dtype=np.float32))
    target = np.ascontiguousarray(np.asarray(target, dtype=np.float32))

    in_maps = [
        {
            "input": np.ascontiguousarray(input[i * BPC:(i + 1) * BPC]),
            "target": np.ascontiguousarray(target[i * BPC:(i + 1) * BPC]),
        }
        for i in range(N_CORES)
    ]
    res = run_bass_kernel_spmd(nc, in_maps, list(range(N_CORES)), trace=TRACE)
    LAST_RESULTS = res

    # host epilogue over tiny per-core partials
    p_grid = np.arange(128)
    cgrid = np.arange(W, dtype=np.float64) + 1.0
    ed = np.zeros((B, C), dtype=np.float64)
    for i in range(N_CORES):
        r = res.results[i]
        psums = np.asarray(r["psums"]).astype(np.float64).reshape(2, HM, W)
        tmax = np.asarray(r["tmax"])
        tidx = np.asarray(r["tidx"])
        for m in range(HM):
            b, ch = divmod(m, C)
            gb = i * BPC + b
            cs = psums[0, m]
            yw = psums[1, m]
            s = cs.sum()
            pred_x = (cs * cgrid).sum() / s
            pred_y = yw.sum() / s
            # hard argmax of target (first-occurrence tie-break on flat index)
            vals = tmax[:, m]
            idx = tidx[:, m].astype(np.int64)
            rows = p_grid * RPP + idx // W
            cols = idx % W
            flat = rows * W + cols
            cand = np.flatnonzero(vals == vals.max())
            w = cand[np.argmin(flat[cand])]
            true_x = float(cols[w] + 1)
            true_y = float(rows[w] + 1)
            ed[gb, ch] = np.sqrt((true_x - pred_x) ** 2 + (true_y - pred_y) ** 2)

    s_i = ed[:, 0].sum()
    s_s = ed[:, 1:].sum()
    return (
        np.float32(s_i / B),
        np.float32(s_s / B),
        np.float32((s_i + s_s) / B),
    )
